# revision 6
# baseline (speedup 1.0000x reference)
"""GRPO loss kernel for Trainium2, 8 NeuronCores.

Strategy (tensor-parallel over vocab):
  - Each core r owns a 4000-row vocab shard of the LM head weight
    (padded to 4096 so PSUM tiles stay 512-aligned; the 96 zero rows
    contribute exp(0)=1 each to the sumexp and are subtracted off).
  - Each core computes, for ALL 8192 tokens, the partial
    sum_{v in shard} exp(h_t . w_v)  via bf16 matmuls (fp32 PSUM accum)
    and a fused Exp+accumulate on the scalar engine.
  - The target logit h_t . w_{tgt_t} is computed exactly once per token,
    token-sharded across cores (1024 tokens each) as a DVE
    multiply+reduce over rows of W gathered by target index on the host.
  - Host combines: logz = log(sum_r S_r), pi_lp = tgt_dot - logz, then
    the tiny [4,2048] GRPO epilogue in numpy float32.

Layouts are pre-tiled on the host so every device DMA is contiguous:
  ht  [64, 128, 16, 128] bf16 : (m, p, ko, t) = H[m*128+t, ko*128+p]
  wt  [128, 16, 4096]    bf16 : (p, ko, v)    = Wshard[v, ko*128+p]
  wtg [8, 128, 2048]     bf16 : gathered W rows for this core's tokens
  hrw [8, 128, 2048]     bf16 : H rows for this core's tokens
"""

import os
import sys

sys.path.insert(0, "/opt/trn_rl_repo")

import numpy as np
import ml_dtypes

import concourse.bass as bass
import concourse.mybir as mybir
import concourse.tile as tile
from concourse import bacc
from concourse.bass_utils import run_bass_kernel_spmd

NCORES = 8
VOCAB, EMB = 32000, 2048
BSZ, SEQ = 4, 2048
TOK = BSZ * SEQ              # 8192
VS = VOCAB // NCORES         # 4000 vocab rows per core
VSP = 4096                   # padded shard (8 x 512)
PAD = VSP - VS               # 96 zero rows -> exp(0)=1 each
TOKS = TOK // NCORES         # 1024 tokens per core for the target dot
KO = EMB // 128              # 16 contraction chunks
M_TILES = TOK // 128         # 64 token tiles
KL_COEFF = 0.1

BF16 = ml_dtypes.bfloat16

_NC_CACHE = {}
_TRACE = [False]
LAST_INFO = {}


def set_trace(flag: bool):
    _TRACE[0] = bool(flag)


def _build_nc(m_tiles=M_TILES, d_tiles=TOKS // 128):
    nc = bacc.Bacc()
    ht = nc.declare_dram_parameter(
        "ht", [m_tiles, 128, KO, 128], mybir.dt.bfloat16, isOutput=False)
    wt = nc.declare_dram_parameter(
        "wt", [128, KO, VSP], mybir.dt.bfloat16, isOutput=False)
    wtg = nc.declare_dram_parameter(
        "wtg", [d_tiles, 128, EMB], mybir.dt.bfloat16, isOutput=False)
    hrw = nc.declare_dram_parameter(
        "hrw", [d_tiles, 128, EMB], mybir.dt.bfloat16, isOutput=False)
    s_out = nc.declare_dram_parameter(
        "s_out", [m_tiles, 128, 1], mybir.dt.float32, isOutput=True)
    d_out = nc.declare_dram_parameter(
        "d_out", [d_tiles, 128, 1], mybir.dt.float32, isOutput=True)

    f32 = mybir.dt.float32
    bf16 = mybir.dt.bfloat16
    Exp = mybir.ActivationFunctionType.Exp

    with tile.TileContext(nc) as tc:
        from contextlib import ExitStack
        with ExitStack() as ctx:
            wpool = ctx.enter_context(tc.tile_pool(name="w", bufs=1))
            hpool = ctx.enter_context(tc.tile_pool(name="h", bufs=3))
            psum = ctx.enter_context(tc.tile_pool(name="ps", bufs=2, space="PSUM"))
            spool = ctx.enter_context(tc.tile_pool(name="scr", bufs=2))
            apool = ctx.enter_context(tc.tile_pool(name="acc", bufs=4))
            opool = ctx.enter_context(tc.tile_pool(name="o", bufs=4))

            # resident weight shard: 16 tiles of [128, 4096] bf16 (16.8 MB)
            w_sb = []
            for k in range(KO):
                t = wpool.tile([128, VSP], bf16, tag=f"w{k}")
                nc.sync.dma_start(out=t[:], in_=wt[:, k, :])
                w_sb.append(t)

            for m in range(m_tiles):
                ht_sb = hpool.tile([128, KO, 128], bf16, tag="ht")
                nc.sync.dma_start(out=ht_sb[:], in_=ht[m])
                accs = []
                for vh in range(2):
                    ps = psum.tile([128, 2048], f32, tag="ps")
                    for k in range(KO):
                        lhsT = ht_sb[:, k, :]
                        for vi in range(4):
                            v0 = vh * 2048 + vi * 512
                            nc.tensor.matmul(
                                ps[:, vi * 512:(vi + 1) * 512],
                                lhsT,
                                w_sb[k][:, v0:v0 + 512],
                                start=(k == 0),
                                stop=(k == KO - 1),
                            )
                    scr = spool.tile([128, 2048], f32, tag="scr")
                    acc = apool.tile([128, 1], f32, tag=f"acc{vh}")
                    nc.scalar.activation(
                        out=scr[:], in_=ps[:], func=Exp, accum_out=acc[:])
                    accs.append(acc)
                s_m = opool.tile([128, 1], f32, tag="s_m")
                nc.vector.tensor_add(s_m[:], accs[0][:], accs[1][:])
                nc.vector.tensor_scalar_add(s_m[:], s_m[:], float(-PAD))
                nc.sync.dma_start(out=s_out[m], in_=s_m[:])

            # target-logit dot products for this core's token slice
            for i in range(d_tiles):
                wtg_sb = hpool.tile([128, EMB], bf16, tag="wtg")
                hrw_sb = hpool.tile([128, EMB], bf16, tag="hrw")
                nc.sync.dma_start(out=wtg_sb[:], in_=wtg[i])
                nc.sync.dma_start(out=hrw_sb[:], in_=hrw[i])
                dscr = spool.tile([128, EMB], f32, tag="scr")
                d_i = opool.tile([128, 1], f32, tag="d_i")
                nc.vector.tensor_mul(dscr[:, :EMB], wtg_sb[:], hrw_sb[:])
                nc.vector.reduce_sum(
                    d_i[:], dscr[:, :EMB], axis=mybir.AxisListType.X)
                nc.sync.dma_start(out=d_out[i], in_=d_i[:])
    nc.compile()
    return nc


def _get_nc():
    key = (M_TILES, TOKS // 128)
    if key not in _NC_CACHE:
        _NC_CACHE[key] = _build_nc(*key)
    return _NC_CACHE[key]


def _prep_inputs(weight, outputs, targets):
    """Host-side shard + layout prep. Returns in_maps for the 8 cores."""
    wbf = np.asarray(weight, dtype=np.float32).astype(BF16)      # [32000, 2048]
    h2 = np.asarray(outputs, dtype=np.float32).reshape(TOK, EMB)
    hbf = h2.astype(BF16)                                        # [8192, 2048]
    tgt = np.asarray(targets).reshape(TOK).astype(np.int64)

    # (m, t, ko, p) -> (m, p, ko, t)
    ht_tiled = np.ascontiguousarray(
        hbf.reshape(M_TILES, 128, KO, 128).transpose(0, 3, 2, 1))

    # (r, v, ko, p) -> (r, p, ko, v), padded on v
    w_t = np.ascontiguousarray(
        wbf.reshape(NCORES, VS, KO, 128).transpose(0, 3, 2, 1))  # [8,128,16,4000]
    wt_tiled = np.zeros((NCORES, 128, KO, VSP), dtype=BF16)
    wt_tiled[:, :, :, :VS] = w_t

    wtg_full = wbf[tgt]                                          # [8192, 2048]

    in_maps = []
    for r in range(NCORES):
        in_maps.append({
            "ht": ht_tiled,
            "wt": np.ascontiguousarray(wt_tiled[r].reshape(128, KO, VSP)),
            "wtg": np.ascontiguousarray(
                wtg_full[r * TOKS:(r + 1) * TOKS].reshape(TOKS // 128, 128, EMB)),
            "hrw": np.ascontiguousarray(
                hbf[r * TOKS:(r + 1) * TOKS].reshape(TOKS // 128, 128, EMB)),
        })
    return in_maps


def _epilogue(S, D, ref_logprobs, advantages, padding_masks):
    """Tiny [4,2048] GRPO epilogue in numpy float32 (mirrors reference.py)."""
    S = S.astype(np.float32)
    logz = np.log(S).astype(np.float32)
    pi_lp = (D.astype(np.float32) - logz).reshape(BSZ, SEQ)

    ref_d = np.asarray(ref_logprobs, dtype=np.float32)
    adv = np.asarray(advantages, dtype=np.float32)
    mask = np.asarray(padding_masks).astype(np.float32)

    delta = ref_d - pi_lp
    per_tok_kl = np.exp(delta) - delta - 1.0
    per_tok_pol = np.exp(pi_lp - pi_lp) * adv[:, None]
    per_tok_loss = -(per_tok_pol - KL_COEFF * per_tok_kl)

    cnt = np.clip(mask.sum(axis=1), 1e-9, None)
    loss = np.float32((((per_tok_loss * mask).sum(axis=1)) / cnt).mean())
    policy_loss = np.float32((((per_tok_pol * mask).sum(axis=1)) / cnt).mean())
    kl_loss = np.float32((((per_tok_kl * mask).sum(axis=1)) / cnt).mean())
    return (
        np.float32(loss),
        np.float32(policy_loss),
        np.float32(kl_loss),
        np.float32(1.0),
        np.float32(0.0),
        pi_lp.astype(np.float32),
    )


def kernel(weight, outputs, targets, ref_logprobs, advantages, padding_masks):
    in_maps = _prep_inputs(weight, outputs, targets)
    nc = _get_nc()
    try:
        res = run_bass_kernel_spmd(
            nc, in_maps, core_ids=list(range(NCORES)), trace=_TRACE[0])
    except ModuleNotFoundError:
        # NTFF profile hook unavailable in this container; run untraced.
        res = run_bass_kernel_spmd(
            nc, in_maps, core_ids=list(range(NCORES)), trace=False)
    LAST_INFO["exec_time_ns"] = res.exec_time_ns
    results = res.results

    S = np.zeros(TOK, dtype=np.float64)
    for r in range(NCORES):
        S += results[r]["s_out"].reshape(TOK).astype(np.float64)
    S = S.astype(np.float32)
    D = np.concatenate(
        [results[r]["d_out"].reshape(TOKS) for r in range(NCORES)])
    return _epilogue(S, D, ref_logprobs, advantages, padding_masks)
